# revision 1
# baseline (speedup 1.0000x reference)
"""Trainium2 Bass kernel for the LDE1D vq_codebook problem.

Math (per batch b):
    q[t,k]   = 2*s0 * x[t,:] @ mu[k,:]          (PE, bf16 in / fp32 accum)
    p[t,k]   = exp(q[t,k])                      (ACT)
    pu[t,k]  = p[t,k] * u[k],  u = exp(-s0*||mu_k||^2)   (DVE ttr)
    D[t]     = sum_k pu[t,k]                    (same DVE ttr, accum_out)
    w[t,k]   = pu[t,k] * weights[t] / D[t]      (DVE)
    acc[k,:] = sum_t w[t,k] * [x[t,:], 1]       (PE, accumulated in PSUM)
    e[k,d]   = acc[k,d] / acc[k,D] - mu[k,d]
Softmax shift-invariance: the -s0*||x||^2 term is constant over k and drops.
The per-k factor u[k] cancels in acc[k,d]/acc[k,D], so using pu instead of
the exact softmax numerator is algebraically exact.

Sharding: data-parallel over B across 8 cores (8 batches each), mu/s
replicated. exp args are bounded (~|2*x.mu| <= ~20) so no max-subtract.
"""

import sys
from contextlib import ExitStack

import numpy as np

sys.path.insert(0, "/opt/trn_rl_repo")

import ml_dtypes

import concourse.bass as bass
import concourse.tile as tile
from concourse import bacc, mybir
from concourse.bass_utils import run_bass_kernel_spmd

BF16 = mybir.dt.bfloat16
F32 = mybir.dt.float32

B, T, D, K = 64, 4096, 256, 64
NCORES = 8
BPC = B // NCORES  # batches per core
TT = 128           # tokens per tile (partition dim)


def build_program(bpc=BPC, t=T, trn_type="TRN2"):
    ntiles = t // TT
    nc = bacc.Bacc(trn_type, target_bir_lowering=False, debug=False,
                   num_devices=NCORES)
    x_d = nc.dram_tensor("x", [bpc, t, D], F32, kind="ExternalInput").ap()
    wsT_d = nc.dram_tensor("wsT", [bpc, TT, ntiles], F32,
                           kind="ExternalInput").ap()
    muT2_d = nc.dram_tensor("muT2", [128, 2 * K], BF16,
                            kind="ExternalInput").ap()
    urep_d = nc.dram_tensor("urep", [128, K], BF16, kind="ExternalInput").ap()
    mu_d = nc.dram_tensor("mu", [K, D], F32, kind="ExternalInput").ap()
    ident_d = nc.dram_tensor("ident", [128, 128], BF16,
                             kind="ExternalInput").ap()
    out_d = nc.dram_tensor("out", [bpc, K, D], F32, kind="ExternalOutput").ap()

    with tile.TileContext(nc) as tc, ExitStack() as ctx:
        _body(ctx, tc, out_d, x_d, wsT_d, muT2_d, urep_d, mu_d, ident_d,
              bpc, ntiles)
    nc.compile()
    return nc


def _body(ctx, tc, out_d, x_d, wsT_d, muT2_d, urep_d, mu_d, ident_d,
          bpc, ntiles):
    nc = tc.nc
    const = ctx.enter_context(tc.tile_pool(name="const", bufs=1))
    muT2 = const.tile([128, 2 * K], BF16)
    nc.sync.dma_start(muT2[:], muT2_d[:])
    urep = const.tile([128, K], BF16)
    nc.sync.dma_start(urep[:], urep_d[:])
    mu_sb = const.tile([K, D], F32)
    nc.sync.dma_start(mu_sb[:], mu_d[:])
    ident = const.tile([128, 128], BF16)
    nc.sync.dma_start(ident[:], ident_d[:])

    xin_pool = ctx.enter_context(tc.tile_pool(name="xin", bufs=4))
    xbf_pool = ctx.enter_context(tc.tile_pool(name="xbf", bufs=4))
    xt_pool = ctx.enter_context(tc.tile_pool(name="xt", bufs=3))
    p_pool = ctx.enter_context(tc.tile_pool(name="p", bufs=3))
    pu_pool = ctx.enter_context(tc.tile_pool(name="pu", bufs=3))
    w_pool = ctx.enter_context(tc.tile_pool(name="w", bufs=3))
    sc_pool = ctx.enter_context(tc.tile_pool(name="sc", bufs=4))
    ws_pool = ctx.enter_context(tc.tile_pool(name="ws", bufs=2))
    res_pool = ctx.enter_context(tc.tile_pool(name="res", bufs=2))
    pt_psum = ctx.enter_context(tc.tile_pool(name="pt", bufs=2, space="PSUM"))
    pq_psum = ctx.enter_context(tc.tile_pool(name="pq", bufs=2, space="PSUM"))
    pe_psum = ctx.enter_context(tc.tile_pool(name="pe", bufs=2, space="PSUM"))

    for b in range(bpc):
        ws = ws_pool.tile([TT, ntiles], F32)
        nc.sync.dma_start(ws[:], wsT_d[b])
        acc = pe_psum.tile([K, D + 1], F32)
        for ti in range(ntiles):
            # load + cast
            xin = xin_pool.tile([TT, D], F32)
            nc.sync.dma_start(xin[:], x_d[b, ti * TT:(ti + 1) * TT, :])
            xbf = xbf_pool.tile([TT, D + 1], BF16)
            nc.gpsimd.tensor_copy(xbf[:, 0:D], xin[:])
            nc.gpsimd.memset(xbf[:, D:D + 1], 1.0)
            # transpose x tile (two 128x128 halves) via PE
            pt = pt_psum.tile([128, D], BF16)
            nc.tensor.transpose(pt[:, 0:128], xbf[:, 0:128], ident[:])
            nc.tensor.transpose(pt[:, 128:256], xbf[:, 128:256], ident[:])
            xt = xt_pool.tile([128, D], BF16)
            nc.scalar.copy(xt[:], pt[:])
            # q = x @ (2 s0 mu)^T : contract d in two halves
            pq = pq_psum.tile([TT, K], F32)
            nc.tensor.matmul(pq[:], xt[:, 0:128], muT2[:, 0:K],
                             start=True, stop=False)
            nc.tensor.matmul(pq[:], xt[:, 128:256], muT2[:, K:2 * K],
                             start=False, stop=True)
            # p = exp(q)  (bf16 out)
            p = p_pool.tile([TT, K], BF16)
            nc.scalar.activation(p[:], pq[:], mybir.ActivationFunctionType.Exp)
            # pu = p * u ;  D_t = sum_k pu
            pu = pu_pool.tile([TT, K], BF16)
            nc.vector.tensor_mul(pu[:], p[:], urep[:])
            dt = sc_pool.tile([TT, 1], F32, tag="dt")
            nc.vector.reduce_sum(dt[:], pu[:], axis=mybir.AxisListType.X)
            # scale_t = weights_t / D_t ; w = pu * scale_t
            rd = sc_pool.tile([TT, 1], F32, tag="rd")
            nc.vector.reciprocal(rd[:], dt[:])
            scl = sc_pool.tile([TT, 1], F32, tag="scl")
            nc.vector.tensor_tensor(scl[:], ws[:, ti:ti + 1], rd[:],
                                    mybir.AluOpType.mult)
            w = w_pool.tile([TT, K], BF16)
            nc.vector.tensor_scalar_mul(w[:], pu[:], scl[:])
            # acc[k, 0:D] += w^T x ; acc[k, D] += w^T 1
            nc.tensor.matmul(acc[:], w[:], xbf[:],
                             start=(ti == 0), stop=(ti == ntiles - 1))
        # epilogue: e = acc[:, :D]/acc[:, D] - mu
        rn = sc_pool.tile([K, 1], F32, tag="rn")
        nc.vector.reciprocal(rn[:], acc[:, D:D + 1])
        ex = res_pool.tile([K, D], F32, tag="ex")
        nc.vector.tensor_scalar_mul(ex[:], acc[:, 0:D], rn[:])
        res = res_pool.tile([K, D], F32, tag="res")
        nc.vector.tensor_sub(res[:], ex[:], mu_sb[:])
        nc.sync.dma_start(out_d[b], res[:])


def make_inputs(x, weights, mu, s, bpc=BPC, t=T):
    """Host-side prep: shard + precompute small replicated tensors."""
    ntiles = t // TT
    s = np.asarray(s, dtype=np.float32)
    s0 = float(s[0])
    if not np.allclose(s, s0):
        raise NotImplementedError("kernel assumes uniform s (as in setup)")
    mu = np.ascontiguousarray(mu, dtype=np.float32)
    mu2t = (2.0 * s0 * mu).T.astype(ml_dtypes.bfloat16)      # [D, K]
    muT2 = np.concatenate([mu2t[:128], mu2t[128:]], axis=1)  # [128, 2K]
    c = s0 * np.sum(mu.astype(np.float64) ** 2, axis=1)
    u = np.exp(-c).astype(ml_dtypes.bfloat16)                # [K]
    urep = np.broadcast_to(u, (128, K)).copy()
    ident = np.eye(128, dtype=ml_dtypes.bfloat16)
    ncores = x.shape[0] // bpc
    in_maps = []
    for ci in range(ncores):
        xs = np.ascontiguousarray(x[ci * bpc:(ci + 1) * bpc, :t],
                                  dtype=np.float32)
        wsl = weights[ci * bpc:(ci + 1) * bpc, :t].astype(np.float32)
        wsT = np.ascontiguousarray(
            wsl.reshape(bpc, ntiles, TT).transpose(0, 2, 1))  # [bpc,128,nt]
        in_maps.append({
            "x": xs, "wsT": wsT, "muT2": muT2, "urep": urep,
            "mu": mu, "ident": ident,
        })
    return in_maps


_CACHE = {}


def _get_program():
    if "nc" not in _CACHE:
        _CACHE["nc"] = build_program()
    return _CACHE["nc"]


def kernel(x, weights, mu, s):
    x = np.asarray(x)
    weights = np.asarray(weights)
    mu = np.asarray(mu, dtype=np.float32)
    s = np.asarray(s, dtype=np.float32)
    nc = _get_program()
    in_maps = make_inputs(x, weights, mu, s)
    res = run_bass_kernel_spmd(nc, in_maps, core_ids=list(range(NCORES)))
    outs = [res.results[ci]["out"].reshape(BPC, K * D)
            for ci in range(NCORES)]
    return np.concatenate(outs, axis=0).astype(np.float32)


if __name__ == "__main__":
    rng = np.random.default_rng(0)
    x = rng.standard_normal((B, T, D), dtype=np.float32)
    w = rng.random((B, T), dtype=np.float32)
    mu = (0.1 * rng.standard_normal((K, D))).astype(np.float32)
    s = np.ones((K,), dtype=np.float32)
    out = kernel(x, weights=w, mu=mu, s=s)
    print("out", out.shape, out.dtype)



# revision 29
# speedup vs baseline: 57.9681x; 57.9681x over previous
"""Trainium2 Bass kernel for the LDE1D vq_codebook problem.

Math (per batch b, K=64 components, D=256 dims, T=4096 tokens):
    q[t,k]   = 2*s0 * x[t,:] @ mu[k,:]            (PE, bf16)
    p[t,k]   = exp(q[t,k])                        (ACT, one op per 8 tiles)
    pu[t,k]  = p[t,k]*u[k]; D[t] = sum_k pu[t,k]  (DVE, batched per 8 tiles)
    w[t,k]   = pu[t,k] * weights[t] / D[t]        (DVE recip + GPSIMD mul)
    acc[k,:] = sum_t w[t,k] * [x[t,:], 1]         (PE, PSUM-accumulated)
    e[k,d]   = acc[k,d] / acc[k,D] - mu[k,d]
Softmax shift-invariance drops the -s0*||x||^2 term, so no max-subtract
(|2 x.mu| <= ~20 is safely inside exp range).

Key implementation points:
  - Host packs xpack[b,t,:] = [bf16(x), 1.0, pad] (258 cols) so one 1MB
    DMA per half-batch delivers x plus the ones column used to fold the
    per-component weight-sum into the second matmul (column D of acc).
  - x tiles are transposed on the PE (transpose-mode matmuls, 16 per
    8-tile supertile, batched into PSUM banks), then copied PSUM->SBUF
    split across ACT and DVE.  The copies move bf16 pairs bitcast as
    f32 words, halving element count (safe: all values finite bf16).
  - Emission is software-pipelined: per step i the PE receives
    transposes(i), q-matmuls(i-1), acc-matmuls(i-3), so the strict-FIFO
    PE never stalls on the lane-engine softmax chain.
  - All lane-engine work is batched at supertile granularity (one exp,
    one pu-mul, one reduce, one reciprocal, one GPSIMD w-mul with a
    stride-0 broadcast of the per-token scale) to amortize sequencer
    overhead and DVE drains.
  - repeat>1 wraps the whole body in a hardware For_i loop; used by the
    timing harness to amortize the ~100ms axon-tunnel dispatch cost.

Sharding: data-parallel over batch B across 8 cores (8 batches each),
mu/s replicated.  The kernel assumes uniform s (as in setup_inputs).
"""

import sys
from contextlib import ExitStack

import numpy as np

sys.path.insert(0, "/opt/trn_rl_repo")

import ml_dtypes

import concourse.bass as bass
import concourse.tile as tile
from concourse import bacc, mybir
from concourse.bass_utils import run_bass_kernel_spmd

BF16 = mybir.dt.bfloat16
F32 = mybir.dt.float32
AF = mybir.ActivationFunctionType
ALU = mybir.AluOpType

B, T, D, K = 64, 4096, 256, 64
NCORES = 8
BPC = B // NCORES   # batches per core
TT = 128            # tokens per tile (partition dim)
XC = D + 2          # packed row: [x(256), 1.0, weight] = 258 cols
HB = 2048           # tokens per DMA (half batch, 1MB bf16)
NHB = T // HB       # DMAs per batch
TPH = HB // TT      # tiles per half-batch (16)
ST = 8              # tiles per supertile
NST = TPH // ST     # supertiles per half-batch (4)


def build_program(bpc=BPC, t=T, repeat=1, passes=None, trn_type="TRN2"):
    nc = bacc.Bacc(trn_type, target_bir_lowering=False, debug=False,
                   num_devices=NCORES)
    xp_d = nc.dram_tensor("xp", [bpc, t, XC], BF16, kind="ExternalInput").ap()
    wsT_d = nc.dram_tensor("wsT", [bpc, TT, t // TT], F32,
                           kind="ExternalInput").ap()
    muT2_d = nc.dram_tensor("muT2", [128, 2 * K], BF16,
                            kind="ExternalInput").ap()
    urep_d = nc.dram_tensor("urep", [128, ST * K], BF16,
                            kind="ExternalInput").ap()
    mu_d = nc.dram_tensor("mu", [K, D], F32, kind="ExternalInput").ap()
    ident_d = nc.dram_tensor("ident", [128, 128], BF16,
                             kind="ExternalInput").ap()
    out_d = nc.dram_tensor("out", [bpc, K, D], F32, kind="ExternalOutput").ap()

    with tile.TileContext(nc) as tc, ExitStack() as ctx:
        _body(ctx, tc, out_d, xp_d, wsT_d, muT2_d, urep_d, mu_d, ident_d,
              bpc, repeat, passes)
    nc.compile()
    return nc


def _emit_core(tc, out_d, xp_d, wsT_d, muT2, urep, mu_sb, ident, pools, bpc):
    """Software-pipelined emission: per loop step i emit
        stage_T(i): dma (on hb start) + transposes + PSUM->SBUF copies
        stage_Q(i-1): q matmuls + exp + pu/D_t + w
        stage_A(i-2): acc matmuls (+ batch epilogue)
    so the PE (strict FIFO) never waits on the lane-engine chain."""
    nc = tc.nc
    (xp_pool, xt_pool, p_pool, pu_pool, w_pool, sc_pool, res_pool, ws_pool,
     pt_psum, pq_psum, pe_psum) = pools

    work = [(b, hb, st) for b in range(bpc) for hb in range(NHB)
            for st in range(NST)]
    xp_cur = {}     # hb-unit -> xp tile
    acc_cur = {}    # b -> acc tile
    state = {}      # i -> per-supertile tiles

    def stage_T(i):
        b, hb, st = work[i]
        if st == 0:
            xp = xp_pool.tile([TT, TPH, XC], BF16, tag="xp")
            src = xp_d[b, hb * HB:(hb + 1) * HB, :].rearrange(
                "(j p) c -> p j c", p=TT)
            if (b, hb) == (0, 0):
                q = TPH // 4
                for piece in range(4):
                    nc.sync.dma_start(xp[:, piece * q:(piece + 1) * q, :],
                                      src[:, piece * q:(piece + 1) * q, :])
            else:
                nc.sync.dma_start(xp[:], src)
            xp_cur[(b, hb)] = xp
        xp = xp_cur[(b, hb)]
        t0 = st * ST
        pt4 = pt_psum.tile([TT, 2 * ST, 128], BF16, tag="pt")
        for j in range(ST):
            for h in range(2):
                jj = 2 * j + h
                nc.tensor.matmul(
                    pt4[:, jj, :], xp[:, t0 + j, h * 128:(h + 1) * 128],
                    ident[:], is_transpose=True,
                    start=(jj % 8 == 0), stop=(jj % 8 == 7))
        xt4 = xt_pool.tile([TT, 2 * ST, 128], BF16, tag="xt")
        # copy bf16 pairs as f32 words: halves ACT element count; all
        # values are finite bf16 (never inf/nan) so bit patterns are safe
        ptf = pt4.bitcast(F32)
        xtf = xt4.bitcast(F32)
        nc.scalar.copy(xtf[:, 0:10, :], ptf[:, 0:10, :])
        nc.vector.tensor_copy(xtf[:, 10:2 * ST, :], ptf[:, 10:2 * ST, :])
        state[i] = {"xp": xp, "xt4": xt4}

    def stage_Q(i):
        b, hb, st = work[i]
        s = state[i]
        xt4 = s["xt4"]
        pq4 = pq_psum.tile([TT, ST * K], F32, tag="pq")
        for j in range(ST):
            for h in range(2):
                nc.tensor.matmul(
                    pq4[:, j * K:(j + 1) * K], xt4[:, 2 * j + h, :],
                    muT2[:, h * K:(h + 1) * K],
                    start=(j == 0 and h == 0), stop=(j == ST - 1 and h == 1))
        p4 = p_pool.tile([TT, ST * K], BF16, tag="p")
        nc.scalar.activation(p4[:], pq4[:, 0:ST * K], AF.Exp)
        pu4 = pu_pool.tile([TT, ST, K], BF16, tag="pu")
        nc.vector.tensor_mul(pu4.rearrange("p j k -> p (j k)"), p4[:],
                             urep[:])
        dt4 = sc_pool.tile([TT, ST], F32, tag="dt")
        nc.vector.reduce_sum(dt4[:], pu4[:], axis=mybir.AxisListType.X)
        rd4 = sc_pool.tile([TT, ST], F32, tag="rd")
        nc.vector.reciprocal(rd4[:], dt4[:])
        scl4 = sc_pool.tile([TT, ST], F32, tag="scl")
        ws = ws_pool[b]
        t0 = st * ST
        tg = hb * TPH + t0
        nc.vector.tensor_tensor(scl4[:], rd4[:], ws[:, tg:tg + ST], ALU.mult)
        w4 = w_pool.tile([TT, ST, K], BF16, tag="w")
        nc.gpsimd.tensor_tensor(
            w4[:], pu4[:],
            scl4[:].unsqueeze(2).broadcast_to([TT, ST, K]), ALU.mult)
        s["w4"] = w4

    def stage_A(i):
        b, hb, st = work[i]
        s = state.pop(i)
        xp, w4 = s["xp"], s["w4"]
        if (hb, st) == (0, 0):
            acc_cur[b] = pe_psum.tile([K, 512], F32, tag="acc", name="acc")
        acc = acc_cur[b]
        t0 = st * ST
        for j in range(ST):
            tg = hb * TPH + t0 + j
            nc.tensor.matmul(acc[:, 0:D + 1], w4[:, j, :],
                             xp[:, t0 + j, 0:D + 1],
                             start=(tg == 0), stop=(tg == NHB * TPH - 1))
        if (hb, st) == (NHB - 1, NST - 1):
            rn = sc_pool.tile([K, 1], F32, tag="rn")
            nc.vector.reciprocal(rn[:], acc[:, D:D + 1])
            ex = res_pool.tile([K, D], F32, tag="ex")
            nc.scalar.activation(ex[:], acc[:, 0:D], AF.Copy, scale=rn[:])
            res = res_pool.tile([K, D], F32, tag="res")
            nc.vector.tensor_sub(res[:], ex[:], mu_sb[:])
            nc.sync.dma_start(out_d[b], res[:])

    n = len(work)
    LAG = 3  # acc stage lag behind transpose stage
    for i in range(n + LAG):
        if i < n:
            stage_T(i)
        if 1 <= i <= n:
            stage_Q(i - 1)
        if i >= LAG:
            stage_A(i - LAG)


def _body(ctx, tc, out_d, xp_d, wsT_d, muT2_d, urep_d, mu_d, ident_d, bpc,
          repeat, passes=None):
    nc = tc.nc
    const = ctx.enter_context(tc.tile_pool(name="const", bufs=1))
    ident = const.tile([128, 128], BF16)
    nc.sync.dma_start(ident[:], ident_d[:])
    muT2 = const.tile([128, 2 * K], BF16)
    nc.sync.dma_start(muT2[:], muT2_d[:])
    urep = const.tile([128, ST * K], BF16)
    nc.sync.dma_start(urep[:], urep_d[:])
    mu_sb = const.tile([K, D], F32)
    nc.sync.dma_start(mu_sb[:], mu_d[:])

    xp_pool = ctx.enter_context(tc.tile_pool(name="xp", bufs=5))
    xt_pool = ctx.enter_context(tc.tile_pool(name="xt", bufs=3))
    p_pool = ctx.enter_context(tc.tile_pool(name="p", bufs=3))
    pu_pool = ctx.enter_context(tc.tile_pool(name="pu", bufs=3))
    w_pool = ctx.enter_context(tc.tile_pool(name="w", bufs=5))
    sc_pool = ctx.enter_context(tc.tile_pool(name="sc", bufs=4))
    res_pool = ctx.enter_context(tc.tile_pool(name="res", bufs=2))
    ws_all = const.tile([TT, bpc, T // TT], F32)
    nc.sync.dma_start(ws_all[:], wsT_d[:].rearrange("b p n -> p b n"))
    ws_pool = [ws_all[:, b, :] for b in range(bpc)]
    pt_psum = ctx.enter_context(tc.tile_pool(name="pt", bufs=2, space="PSUM"))
    pq_psum = ctx.enter_context(tc.tile_pool(name="pq", bufs=2, space="PSUM"))
    pe_psum = ctx.enter_context(tc.tile_pool(name="pe", bufs=2, space="PSUM"))

    pools = (xp_pool, xt_pool, p_pool, pu_pool, w_pool, sc_pool, res_pool,
             ws_pool, pt_psum, pq_psum, pe_psum)

    if passes is None:
        passes = 1
    if repeat == 1:
        _emit_core(tc, out_d, xp_d, wsT_d, muT2, urep, mu_sb, ident, pools,
                   bpc)
    else:
        with tc.For_i(0, repeat):
            for _ in range(passes):
                _emit_core(tc, out_d, xp_d, wsT_d, muT2, urep, mu_sb, ident,
                           pools, bpc)


def make_inputs(x, weights, mu, s, bpc=BPC, t=T):
    """Host-side prep: shard + pack [x|1|w] rows + tiny replicated tensors."""
    s = np.asarray(s, dtype=np.float32)
    s0 = float(s[0])
    if not np.allclose(s, s0):
        raise NotImplementedError("kernel assumes uniform s (as in setup)")
    mu = np.ascontiguousarray(mu, dtype=np.float32)
    mu2t = (2.0 * s0 * mu).T.astype(ml_dtypes.bfloat16)      # [D, K]
    muT2 = np.concatenate([mu2t[:128], mu2t[128:]], axis=1)  # [128, 2K]
    c = s0 * np.sum(mu.astype(np.float64) ** 2, axis=1)
    u = np.exp(-c).astype(ml_dtypes.bfloat16)                # [K]
    urep = np.broadcast_to(np.tile(u, ST), (128, ST * K)).copy()
    ident = np.eye(128, dtype=ml_dtypes.bfloat16)

    ncores = x.shape[0] // bpc
    xpack = np.empty((x.shape[0], t, XC), dtype=ml_dtypes.bfloat16)
    xpack[:, :, 0:D] = x[:, :t].astype(ml_dtypes.bfloat16)
    xpack[:, :, D] = np.asarray(1.0, dtype=ml_dtypes.bfloat16)
    xpack[:, :, D + 1] = weights[:, :t].astype(ml_dtypes.bfloat16)
    ntiles = t // TT
    wsT = np.ascontiguousarray(
        weights[:, :t].astype(np.float32).reshape(-1, ntiles, TT)
        .transpose(0, 2, 1))                                 # [B, 128, nt]
    in_maps = []
    for ci in range(ncores):
        in_maps.append({
            "xp": xpack[ci * bpc:(ci + 1) * bpc],
            "wsT": wsT[ci * bpc:(ci + 1) * bpc],
            "muT2": muT2, "urep": urep, "mu": mu, "ident": ident,
        })
    return in_maps


_CACHE = {}


def _get_program():
    if "nc" not in _CACHE:
        _CACHE["nc"] = build_program()
    return _CACHE["nc"]


def kernel(x, weights, mu, s):
    x = np.asarray(x)
    weights = np.asarray(weights)
    mu = np.asarray(mu, dtype=np.float32)
    s = np.asarray(s, dtype=np.float32)
    nc = _get_program()
    in_maps = make_inputs(x, weights, mu, s)
    res = run_bass_kernel_spmd(nc, in_maps, core_ids=list(range(NCORES)))
    outs = [res.results[ci]["out"].reshape(BPC, K * D)
            for ci in range(NCORES)]
    return np.concatenate(outs, axis=0).astype(np.float32)


if __name__ == "__main__":
    rng = np.random.default_rng(0)
    x = rng.standard_normal((B, T, D), dtype=np.float32)
    w = rng.random((B, T), dtype=np.float32)
    mu = (0.1 * rng.standard_normal((K, D))).astype(np.float32)
    s = np.ones((K,), dtype=np.float32)
    out = kernel(x, weights=w, mu=mu, s=s)
    print("out", out.shape, out.dtype)


# revision 32
# speedup vs baseline: 63.1965x; 1.0902x over previous
"""Trainium2 Bass kernel for the LDE1D vq_codebook problem.

Math (per batch b, K=64 components, D=256 dims, T=4096 tokens):
    q[t,k]   = 2*s0 * x[t,:] @ mu[k,:]            (PE, bf16)
    p[t,k]   = exp(q[t,k])                        (ACT, one op per 8 tiles)
    pu[t,k]  = p[t,k]*u[k]; D[t] = sum_k pu[t,k]  (DVE, batched per 8 tiles)
    w[t,k]   = pu[t,k] * weights[t] / D[t]        (DVE recip + GPSIMD mul)
    acc[k,:] = sum_t w[t,k] * [x[t,:], 1]         (PE, PSUM-accumulated)
    e[k,d]   = acc[k,d] / acc[k,D] - mu[k,d]
Softmax shift-invariance drops the -s0*||x||^2 term, so no max-subtract
(|2 x.mu| <= ~20 is safely inside exp range).

Key implementation points:
  - Host packs xpack[b,t,:] = [bf16(x), 1.0, pad] (258 cols) so one 1MB
    DMA per half-batch delivers x plus the ones column used to fold the
    per-component weight-sum into the second matmul (column D of acc).
  - x tiles are transposed on the PE (transpose-mode matmuls, 16 per
    8-tile supertile, batched into PSUM banks), then copied PSUM->SBUF
    split across ACT and DVE.  The copies move bf16 pairs bitcast as
    f32 words, halving element count (safe: all values finite bf16).
  - Emission is software-pipelined: per step i the PE receives
    transposes(i), q-matmuls(i-1), acc-matmuls(i-4), so the strict-FIFO
    PE never stalls on the lane-engine softmax chain.
  - All lane-engine work is batched at supertile granularity (one exp,
    one pu-mul, one reduce, one reciprocal, one GPSIMD w-mul with a
    stride-0 broadcast of the per-token scale) to amortize sequencer
    overhead and DVE drains.
  - repeat>1 wraps the whole body in a hardware For_i loop; used by the
    timing harness to amortize the ~100ms axon-tunnel dispatch cost.

Sharding: data-parallel over batch B across 8 cores (8 batches each),
mu/s replicated.  The kernel assumes uniform s (as in setup_inputs).
"""

import sys
from contextlib import ExitStack

import numpy as np

sys.path.insert(0, "/opt/trn_rl_repo")

import ml_dtypes

import concourse.bass as bass
import concourse.tile as tile
from concourse import bacc, mybir
from concourse.bass_utils import run_bass_kernel_spmd

BF16 = mybir.dt.bfloat16
F32 = mybir.dt.float32
AF = mybir.ActivationFunctionType
ALU = mybir.AluOpType

B, T, D, K = 64, 4096, 256, 64
NCORES = 8
BPC = B // NCORES   # batches per core
TT = 128            # tokens per tile (partition dim)
XC = D + 2          # packed row: [x(256), 1.0, weight] = 258 cols
HB = 2048           # tokens per DMA (half batch, 1MB bf16)
NHB = T // HB       # DMAs per batch
TPH = HB // TT      # tiles per half-batch (16)
ST = 8              # tiles per supertile
NST = TPH // ST     # supertiles per half-batch (4)


def build_program(bpc=BPC, t=T, repeat=1, passes=None, trn_type="TRN2"):
    nc = bacc.Bacc(trn_type, target_bir_lowering=False, debug=False,
                   num_devices=NCORES)
    xp_d = nc.dram_tensor("xp", [bpc, t, XC], BF16, kind="ExternalInput").ap()
    wsT_d = nc.dram_tensor("wsT", [bpc, TT, t // TT], F32,
                           kind="ExternalInput").ap()
    muT2_d = nc.dram_tensor("muT2", [128, 2 * K], BF16,
                            kind="ExternalInput").ap()
    urep_d = nc.dram_tensor("urep", [128, ST * K], BF16,
                            kind="ExternalInput").ap()
    mu_d = nc.dram_tensor("mu", [K, D], F32, kind="ExternalInput").ap()
    ident_d = nc.dram_tensor("ident", [128, 128], BF16,
                             kind="ExternalInput").ap()
    out_d = nc.dram_tensor("out", [bpc, K, D], F32, kind="ExternalOutput").ap()

    with tile.TileContext(nc) as tc, ExitStack() as ctx:
        _body(ctx, tc, out_d, xp_d, wsT_d, muT2_d, urep_d, mu_d, ident_d,
              bpc, repeat, passes)
    nc.compile()
    return nc


def _emit_core(tc, out_d, xp_d, wsT_d, muT2, urep, mu_sb, ident, pools, bpc):
    """Software-pipelined emission: per loop step i emit
        stage_T(i): dma (on hb start) + transposes + PSUM->SBUF copies
        stage_Q(i-1): q matmuls + exp + pu/D_t + w
        stage_A(i-LAG): acc matmuls (+ batch epilogue)
    so the PE (strict FIFO) never waits on the lane-engine chain."""
    nc = tc.nc
    (xp_pool, xt_pool, p_pool, pu_pool, w_pool, sc_pool, res_pool, ws_pool,
     pt_psum, pq_psum, pe_psum) = pools

    work = [(b, hb, st) for b in range(bpc) for hb in range(NHB)
            for st in range(NST)]
    xp_cur = {}     # hb-unit -> xp tile
    acc_cur = {}    # b -> acc tile
    state = {}      # i -> per-supertile tiles

    def stage_T(i):
        b, hb, st = work[i]
        if st == 0:
            xp = xp_pool.tile([TT, TPH, XC], BF16, tag="xp")
            src = xp_d[b, hb * HB:(hb + 1) * HB, :].rearrange(
                "(j p) c -> p j c", p=TT)
            if (b, hb) == (0, 0):
                q = TPH // 4
                for piece in range(4):
                    nc.sync.dma_start(xp[:, piece * q:(piece + 1) * q, :],
                                      src[:, piece * q:(piece + 1) * q, :])
            else:
                nc.sync.dma_start(xp[:], src)
            xp_cur[(b, hb)] = xp
        xp = xp_cur[(b, hb)]
        t0 = st * ST
        pt4 = pt_psum.tile([TT, 2 * ST, 128], BF16, tag="pt")
        for j in range(ST):
            for h in range(2):
                jj = 2 * j + h
                nc.tensor.matmul(
                    pt4[:, jj, :], xp[:, t0 + j, h * 128:(h + 1) * 128],
                    ident[:], is_transpose=True,
                    start=(jj % 8 == 0), stop=(jj % 8 == 7))
        xt4 = xt_pool.tile([TT, 2 * ST, 128], BF16, tag="xt")
        # copy bf16 pairs as f32 words: halves ACT element count; all
        # values are finite bf16 (never inf/nan) so bit patterns are safe
        ptf = pt4.bitcast(F32)
        xtf = xt4.bitcast(F32)
        nc.scalar.copy(xtf[:, 0:12, :], ptf[:, 0:12, :])
        nc.vector.tensor_copy(xtf[:, 12:2 * ST, :], ptf[:, 12:2 * ST, :])
        state[i] = {"xp": xp, "xt4": xt4}

    def stage_Q(i):
        b, hb, st = work[i]
        s = state[i]
        xt4 = s["xt4"]
        pq4 = pq_psum.tile([TT, ST * K], F32, tag="pq")
        for j in range(ST):
            for h in range(2):
                nc.tensor.matmul(
                    pq4[:, j * K:(j + 1) * K], xt4[:, 2 * j + h, :],
                    muT2[:, h * K:(h + 1) * K],
                    start=(j == 0 and h == 0), stop=(j == ST - 1 and h == 1))
        p4 = p_pool.tile([TT, ST * K], BF16, tag="p")
        nc.scalar.activation(p4[:], pq4[:, 0:ST * K], AF.Exp)
        pu4 = pu_pool.tile([TT, ST, K], BF16, tag="pu")
        nc.vector.tensor_mul(pu4.rearrange("p j k -> p (j k)"), p4[:],
                             urep[:])
        dt4 = sc_pool.tile([TT, ST], F32, tag="dt")
        nc.vector.reduce_sum(dt4[:], pu4[:], axis=mybir.AxisListType.X)
        rd4 = sc_pool.tile([TT, ST], F32, tag="rd")
        nc.vector.reciprocal(rd4[:], dt4[:])
        scl4 = sc_pool.tile([TT, ST], F32, tag="scl")
        ws = ws_pool[b]
        t0 = st * ST
        tg = hb * TPH + t0
        nc.vector.tensor_tensor(scl4[:], rd4[:], ws[:, tg:tg + ST], ALU.mult)
        w4 = w_pool.tile([TT, ST, K], BF16, tag="w")
        nc.gpsimd.tensor_tensor(
            w4[:], pu4[:],
            scl4[:].unsqueeze(2).broadcast_to([TT, ST, K]), ALU.mult)
        s["w4"] = w4

    def stage_A(i):
        b, hb, st = work[i]
        s = state.pop(i)
        xp, w4 = s["xp"], s["w4"]
        if (hb, st) == (0, 0):
            acc_cur[b] = pe_psum.tile([K, 512], F32, tag="acc", name="acc")
        acc = acc_cur[b]
        t0 = st * ST
        for j in range(ST):
            tg = hb * TPH + t0 + j
            nc.tensor.matmul(acc[:, 0:D + 1], w4[:, j, :],
                             xp[:, t0 + j, 0:D + 1],
                             start=(tg == 0), stop=(tg == NHB * TPH - 1))
        if (hb, st) == (NHB - 1, NST - 1):
            rn = sc_pool.tile([K, 1], F32, tag="rn")
            nc.vector.reciprocal(rn[:], acc[:, D:D + 1])
            ex = res_pool.tile([K, D], F32, tag="ex")
            nc.scalar.activation(ex[:], acc[:, 0:D], AF.Copy, scale=rn[:])
            res = res_pool.tile([K, D], F32, tag="res")
            nc.vector.tensor_sub(res[:], ex[:], mu_sb[:])
            nc.sync.dma_start(out_d[b], res[:])

    n = len(work)
    LAG = 4  # acc stage lag behind transpose stage
    for i in range(n + LAG):
        if i < n:
            stage_T(i)
        if 1 <= i <= n:
            stage_Q(i - 1)
        if i >= LAG:
            stage_A(i - LAG)


def _body(ctx, tc, out_d, xp_d, wsT_d, muT2_d, urep_d, mu_d, ident_d, bpc,
          repeat, passes=None):
    nc = tc.nc
    const = ctx.enter_context(tc.tile_pool(name="const", bufs=1))
    ident = const.tile([128, 128], BF16)
    nc.sync.dma_start(ident[:], ident_d[:])
    muT2 = const.tile([128, 2 * K], BF16)
    nc.sync.dma_start(muT2[:], muT2_d[:])
    urep = const.tile([128, ST * K], BF16)
    nc.sync.dma_start(urep[:], urep_d[:])
    mu_sb = const.tile([K, D], F32)
    nc.sync.dma_start(mu_sb[:], mu_d[:])

    xp_pool = ctx.enter_context(tc.tile_pool(name="xp", bufs=5))
    xt_pool = ctx.enter_context(tc.tile_pool(name="xt", bufs=3))
    p_pool = ctx.enter_context(tc.tile_pool(name="p", bufs=3))
    pu_pool = ctx.enter_context(tc.tile_pool(name="pu", bufs=3))
    w_pool = ctx.enter_context(tc.tile_pool(name="w", bufs=5))
    sc_pool = ctx.enter_context(tc.tile_pool(name="sc", bufs=4))
    res_pool = ctx.enter_context(tc.tile_pool(name="res", bufs=2))
    ws_all = const.tile([TT, bpc, T // TT], F32)
    nc.sync.dma_start(ws_all[:], wsT_d[:].rearrange("b p n -> p b n"))
    ws_pool = [ws_all[:, b, :] for b in range(bpc)]
    pt_psum = ctx.enter_context(tc.tile_pool(name="pt", bufs=2, space="PSUM"))
    pq_psum = ctx.enter_context(tc.tile_pool(name="pq", bufs=2, space="PSUM"))
    pe_psum = ctx.enter_context(tc.tile_pool(name="pe", bufs=2, space="PSUM"))

    pools = (xp_pool, xt_pool, p_pool, pu_pool, w_pool, sc_pool, res_pool,
             ws_pool, pt_psum, pq_psum, pe_psum)

    if passes is None:
        passes = 1
    if repeat == 1:
        _emit_core(tc, out_d, xp_d, wsT_d, muT2, urep, mu_sb, ident, pools,
                   bpc)
    else:
        with tc.For_i(0, repeat):
            for _ in range(passes):
                _emit_core(tc, out_d, xp_d, wsT_d, muT2, urep, mu_sb, ident,
                           pools, bpc)


def make_inputs(x, weights, mu, s, bpc=BPC, t=T):
    """Host-side prep: shard + pack [x|1|w] rows + tiny replicated tensors."""
    s = np.asarray(s, dtype=np.float32)
    s0 = float(s[0])
    if not np.allclose(s, s0):
        raise NotImplementedError("kernel assumes uniform s (as in setup)")
    mu = np.ascontiguousarray(mu, dtype=np.float32)
    mu2t = (2.0 * s0 * mu).T.astype(ml_dtypes.bfloat16)      # [D, K]
    muT2 = np.concatenate([mu2t[:128], mu2t[128:]], axis=1)  # [128, 2K]
    c = s0 * np.sum(mu.astype(np.float64) ** 2, axis=1)
    u = np.exp(-c).astype(ml_dtypes.bfloat16)                # [K]
    urep = np.broadcast_to(np.tile(u, ST), (128, ST * K)).copy()
    ident = np.eye(128, dtype=ml_dtypes.bfloat16)

    ncores = x.shape[0] // bpc
    xpack = np.empty((x.shape[0], t, XC), dtype=ml_dtypes.bfloat16)
    xpack[:, :, 0:D] = x[:, :t].astype(ml_dtypes.bfloat16)
    xpack[:, :, D] = np.asarray(1.0, dtype=ml_dtypes.bfloat16)
    xpack[:, :, D + 1] = weights[:, :t].astype(ml_dtypes.bfloat16)
    ntiles = t // TT
    wsT = np.ascontiguousarray(
        weights[:, :t].astype(np.float32).reshape(-1, ntiles, TT)
        .transpose(0, 2, 1))                                 # [B, 128, nt]
    in_maps = []
    for ci in range(ncores):
        in_maps.append({
            "xp": xpack[ci * bpc:(ci + 1) * bpc],
            "wsT": wsT[ci * bpc:(ci + 1) * bpc],
            "muT2": muT2, "urep": urep, "mu": mu, "ident": ident,
        })
    return in_maps


_CACHE = {}


def _get_program():
    if "nc" not in _CACHE:
        _CACHE["nc"] = build_program()
    return _CACHE["nc"]


def kernel(x, weights, mu, s):
    x = np.asarray(x)
    weights = np.asarray(weights)
    mu = np.asarray(mu, dtype=np.float32)
    s = np.asarray(s, dtype=np.float32)
    nc = _get_program()
    in_maps = make_inputs(x, weights, mu, s)
    res = run_bass_kernel_spmd(nc, in_maps, core_ids=list(range(NCORES)))
    outs = [res.results[ci]["out"].reshape(BPC, K * D)
            for ci in range(NCORES)]
    return np.concatenate(outs, axis=0).astype(np.float32)


if __name__ == "__main__":
    rng = np.random.default_rng(0)
    x = rng.standard_normal((B, T, D), dtype=np.float32)
    w = rng.random((B, T), dtype=np.float32)
    mu = (0.1 * rng.standard_normal((K, D))).astype(np.float32)
    s = np.ones((K,), dtype=np.float32)
    out = kernel(x, weights=w, mu=mu, s=s)
    print("out", out.shape, out.dtype)
